# revision 2
# baseline (speedup 1.0000x reference)
"""ConvKAN Trainium2 kernel (v4: bc-major layout, half-width early waves).

Decomposition (validated vs reference):
  out[(b, cin, kh, kw, q), oc] =
      sum_{func, jh, jw} Wf[oc, func, jh*48+jw] * F_func(x_pad[b, cin, 12q+jh+kh, jw+kw])
  where F_0 = silu and F_{1+g}(v) = spline cubes 4*r1^3 - r2^3 with
  t = |2.5 v + 3.5 - g|, r2 = max(2-t, 0), r1 = max(1-t, 0)
  (weights carry the -1/6 normalization).

Sharding: input channels cin split 8 ways (8 per core); core k produces
output rows [288k, 288k+288) of (B, 2304, OUT_C).

v4 layout: free dim is (bc=128, h=50) with h minor, so planes and
matmuls can be split along bc. Early waves (silu, spline pass 0) run
N=256 bc-halves so the PE starts as soon as half a plane is computed.
start=True only on the first matmul touching each PSUM bank; everything
else accumulates (per-element has_written semantics make half-width
accumulation sound).

Inputs land on one HWDGE queue in consumption order:
  xs -> wq[silu] -> xp0 -> wq[p0] -> xp1 -> wq[p1] -> xp2 -> wq[p2].
Group 8 (kh,kw)=(2,2) runs right after group 0's bank drains; last
event is group 7's drain + fp16 output DMA.
"""

from contextlib import ExitStack

import numpy as np

import concourse.bass as bass
import concourse.bacc as bacc
import concourse.tile as tile
from concourse import mybir
from concourse.alu_op_type import AluOpType
from concourse.bass_utils import run_bass_kernel_spmd

AF = mybir.ActivationFunctionType
DT = mybir.dt

B, C, H, W = 16, 64, 48, 48
OUT_C = 128
NCORES = 8
CLOC = C // NCORES          # 8 input channels per core
BC = B * CLOC               # 128 (b, c) pairs per core
HP = 50                     # padded height
FREE = HP * BC              # 6400
NSP = 3                     # spline passes
NTILE = 3 * 6 + NSP * 3 * 12  # 126 lhsT tiles: (silu kw jj) + (pass kw jh)
FCH = 4                     # bc chunks per activation pass (32 b each)
RUN_KWARGS = {}
LAST_EXEC_NS = None
N_WARMUP = 6                # HAM warm-up dummy matmuls

# engine-assignment knobs, [pass][chunk]: square ops on ACT vs DVE
S2_ACT = ((False, False, True, True),
          (True, True, False, False),
          (True, True, False, False))
S1F_ACT = ((True, True, True, True),
           (True, True, True, True),
           (True, True, True, True))
# which waves run as bc-halves (N=256): silu and pass0
HALF_SILU = True
HALF_P0 = True

V0 = (0.0, 0.0, -0.125, -2.875, -2.875, -0.125, 0.0, 0.0)  # slot value at x=0


def build_nc(fch: int = FCH) -> bass.Bass:
    nc = bacc.Bacc(None, target_bir_lowering=False, debug=True)
    xs = nc.declare_dram_parameter("xs", [128, FREE], DT.float16, isOutput=False)
    xp = nc.declare_dram_parameter("xp", [128, NSP * FREE], DT.float16,
                                   isOutput=False)
    wq = nc.declare_dram_parameter("wq", [128, NTILE * 128], DT.float16,
                                   isOutput=False)
    bias = nc.declare_dram_parameter("bias", [128, 8], DT.float32, isOutput=False)
    out = nc.declare_dram_parameter("out", [9, 128, 512], DT.float16, isOutput=True)

    fw = FREE // fch
    with ExitStack() as ctx:
        tc = ctx.enter_context(tile.TileContext(nc))
        wpool = ctx.enter_context(tc.tile_pool(name="w", bufs=1))
        fpool = ctx.enter_context(tc.tile_pool(name="f", bufs=2))
        psum_pool = ctx.enter_context(tc.tile_pool(name="ps", bufs=8, space="PSUM"))
        opool = ctx.enter_context(tc.tile_pool(name="o", bufs=4))

        bias_sb = wpool.tile([128, 8], DT.float32)
        nc.gpsimd.dma_start(bias_sb[:], bias[:])

        xs_sb = wpool.tile([128, FREE], DT.float16)
        wq_sb = wpool.tile([128, NTILE * 128], DT.float16)
        xp_sb = [wpool.tile([128, FREE], DT.float16, name=f"xp{c}", tag=f"xp{c}")
                 for c in range(NSP)]

        # input DMA stream in consumption order on the HWDGE queue
        for f in range(fch):
            sl = slice(f * fw, (f + 1) * fw)
            nc.sync.dma_start(xs_sb[:, sl], xs[:, sl])
        nc.sync.dma_start(wq_sb[:, :18 * 128], wq[:, :18 * 128])  # silu tiles
        for c in range(NSP):
            for f in range(fch):
                nc.sync.dma_start(xp_sb[c][:, f * fw:(f + 1) * fw],
                                  xp[:, c * FREE + f * fw:c * FREE + (f + 1) * fw])
            wsl = slice((18 + c * 36) * 128, (18 + (c + 1) * 36) * 128)
            nc.sync.dma_start(wq_sb[:, wsl], wq[:, wsl])

        ts_s = wpool.tile([128, FREE], DT.float16, name="tsS", tag="tsS")
        ts_t = [wpool.tile([128, FREE], DT.float16, name=f"ts{c}", tag=f"ts{c}")
                for c in range(NSP)]

        groups = [(kh, kw) for kh in range(3) for kw in range(3)]
        ps_tiles = {}
        for g in groups[:8]:
            ps_tiles[g] = psum_pool.tile([128, 512], DT.float32,
                                         name=f"ps_{g[0]}{g[1]}", tag="ps")
        # HAM warm-up into group-7's bank (cleared by its first start=True mm)
        warm = ps_tiles[groups[7]][:]
        for _ in range(N_WARMUP):
            nc.tensor.matmul(warm, xs_sb[:, 0:128], xs_sb[:, 512:1024],
                             start=True, stop=False)

        # silu chain: one ACT op per chunk
        for f in range(fch):
            sl = slice(f * fw, (f + 1) * fw)
            nc.scalar.activation(ts_s[:, sl], xs_sb[:, sl], AF.Silu)

        # spline chains
        for c in range(NSP):
            bias_ap = bias_sb[:, c:c + 1]
            for f in range(fch):
                sl = slice(f * fw, (f + 1) * fw)
                t = fpool.tile([128, fw], DT.float16, tag="t")
                nc.scalar.activation(t[:], xp_sb[c][:, sl], AF.Abs,
                                     bias=bias_ap, scale=2.5)
                nr2 = fpool.tile([128, fw], DT.float16, tag="nr2")  # -r2
                nc.vector.tensor_scalar(nr2[:], t[:], 2.0, 0.0,
                                        op0=AluOpType.subtract, op1=AluOpType.min)
                nr1 = fpool.tile([128, fw], DT.float16, tag="nr1")  # -r1
                nc.vector.tensor_scalar(nr1[:], t[:], 1.0, 0.0,
                                        op0=AluOpType.subtract, op1=AluOpType.min)
                s2 = fpool.tile([128, fw], DT.float16, tag="s2")    # r2^2
                if S2_ACT[c][f]:
                    nc.scalar.activation(s2[:], nr2[:], AF.Square)
                else:
                    nc.vector.tensor_tensor(s2[:], nr2[:], nr2[:],
                                            op=AluOpType.mult)
                s1f = fpool.tile([128, fw], DT.float16, tag="s1f")  # 4 r1^2
                if S1F_ACT[c][f]:
                    nc.scalar.activation(s1f[:], nr1[:], AF.Square, scale=2.0)
                else:
                    t4 = fpool.tile([128, fw], DT.float16, tag="t4")
                    nc.vector.tensor_scalar(t4[:], nr1[:], 2.0, 0.0,
                                            op0=AluOpType.mult, op1=AluOpType.bypass)
                    nc.vector.tensor_tensor(s1f[:], t4[:], t4[:],
                                            op=AluOpType.mult)
                c2n = fpool.tile([128, fw], DT.float16, tag="c2n")  # -r2^3
                nc.vector.tensor_tensor(c2n[:], s2[:], nr2[:], op=AluOpType.mult)
                cn1 = fpool.tile([128, fw], DT.float16, tag="cn1")  # -4 r1^3
                nc.vector.tensor_tensor(cn1[:], s1f[:], nr1[:], op=AluOpType.mult)
                nc.vector.tensor_tensor(ts_t[c][:, sl], c2n[:], cn1[:],
                                        op=AluOpType.subtract)

        def emit_mm(g, seq, b0=0, b1=BC, start=False, stop=False):
            kh, kw = g
            kind, c, j = seq
            if kind == "S":
                idx = kw * 6 + j
                src = ts_s
            else:
                idx = 18 + (c * 3 + kw) * 12 + j
                src = ts_t[c]
            lhsT = wq_sb[:, idx * 128:(idx + 1) * 128]
            h0 = kh + j
            rhs = src[:].rearrange("p (b h) -> p b h", h=HP)[:, b0:b1,
                                                            h0:h0 + 37:12]
            ps3 = ps_tiles[g][:].rearrange("p (b q) -> p b q", q=4)[:, b0:b1, :]
            nc.tensor.matmul(ps3, lhsT, rhs, start=start, stop=stop)

        def drain(g):
            ob = opool.tile([128, 512], DT.float16)
            # adds the constant contribution of the removed w_pad 0/49 slots
            nc.scalar.activation(ob[:], ps_tiles[g][:], AF.Identity,
                                 bias=bias_sb[:, 4 + g[1]:5 + g[1]])
            nc.sync.dma_start(out[g[0] * 3 + g[1]], ob[:])

        wave = groups[:8]
        silu_seqs = [("S", 0, j) for j in range(6)]
        pass_seqs = [[("P", c, j) for j in range(12)] for c in range(NSP)]
        all_seqs = silu_seqs + pass_seqs[0] + pass_seqs[1] + pass_seqs[2]

        # silu wave (bc-halves so it can start on half a plane)
        silu_halves = [(0, BC // 2), (BC // 2, BC)] if HALF_SILU else [(0, BC)]
        for hi, (b0, b1) in enumerate(silu_halves):
            for j, s in enumerate(silu_seqs):
                for g in wave:
                    emit_mm(g, s, b0, b1, start=(hi == 0 and j == 0))
        # pass 0 (bc-halves), passes 1-2 full width
        p0_halves = [(0, BC // 2), (BC // 2, BC)] if HALF_P0 else [(0, BC)]
        for b0, b1 in p0_halves:
            for s in pass_seqs[0]:
                for g in wave:
                    emit_mm(g, s, b0, b1)
        for s in pass_seqs[1]:
            for g in wave:
                emit_mm(g, s)
        # pass 2: group 0 first, drain it, then g8's full run, then the rest
        for i, s in enumerate(pass_seqs[2]):
            emit_mm(wave[0], s, stop=(i == 11))
        drain(wave[0])
        g8 = groups[8]
        ps_tiles[g8] = psum_pool.tile([128, 512], DT.float32, name="ps_22",
                                      tag="ps")
        for i, s in enumerate(all_seqs):
            emit_mm(g8, s, start=(i == 0), stop=(i == len(all_seqs) - 1))
        drain(g8)
        for g in wave[1:]:
            for i, s in enumerate(pass_seqs[2]):
                emit_mm(g, s, stop=(i == 11))
            drain(g)
    nc.compile()
    return nc


def _prep_weights(base_weight, spline_weight, spline_scaler):
    # Wf[oc, func, jj]: func 0 = silu weights, 1+g = scaled spline / -6
    wf = np.empty((OUT_C, 9, 576), dtype=np.float64)
    wf[:, 0, :] = base_weight
    wf[:, 1:, :] = np.moveaxis(
        spline_weight.astype(np.float64)
        * spline_scaler.astype(np.float64)[..., None] / -6.0, -1, 1)
    w4 = wf.reshape(OUT_C, 9, 12, 48)
    wq = np.zeros((128, NTILE, OUT_C), dtype=np.float64)
    for kw in range(3):
        for jj in range(6):  # silu tiles
            idx = kw * 6 + jj
            for p in range(96):
                s, wp = p // 48, 1 + p % 48
                jw = wp - kw
                if 0 <= jw < 48:
                    wq[p, idx, :] = w4[:, 0, jj + 6 * s, jw]
    for c in range(NSP):
        for kw in range(3):
            for jh in range(12):
                idx = 18 + (c * 3 + kw) * 12 + jh
                for p in range(128):
                    flat = 128 * c + p
                    g, wp = flat // 48, 1 + flat % 48
                    jw = wp - kw
                    if 0 <= jw < 48:
                        wq[p, idx, :] = w4[:, 1 + g, jh, jw]
    wq = wq.reshape(128, NTILE * 128).astype(np.float16)

    bias = np.zeros((128, 8), dtype=np.float32)
    for c in range(NSP):
        for p in range(128):
            bias[p, c] = 3.5 - (128 * c + p) // 48
    # drain-time constant for removed w_pad 0 (kw=0) / 49 (kw=2) slots
    for g in range(8):
        bias[:, 4] += V0[g] * w4[:, 1 + g, :, 0].sum(axis=1)
        bias[:, 6] += V0[g] * w4[:, 1 + g, :, 47].sum(axis=1)
    return wq, bias


def _prep_x(x_slice):
    # x_slice: (B, CLOC, 48, 48) -> (xs [128, FREE], xp [128, NSP*FREE]) fp16
    # free dim layout: col = b*50 + h  (bc major, padded h minor)
    plane = np.zeros((48, BC, HP), dtype=np.float32)  # [w, b, h_pad]
    plane[:, :, 1:49] = np.ascontiguousarray(
        x_slice.transpose(3, 0, 1, 2)).reshape(48, BC, 48)
    sh6 = np.zeros_like(plane)              # h-shift by 6
    sh6[:, :, 0:44] = plane[:, :, 6:50]

    xs = np.zeros((128, FREE), dtype=np.float16)
    xs[0:48] = plane.reshape(48, FREE)
    xs[48:96] = sh6.reshape(48, FREE)
    xp = np.empty((128, NSP * FREE), dtype=np.float16)
    flat = plane.reshape(48, FREE)
    for c in range(NSP):
        rows = [(128 * c + p) % 48 for p in range(128)]
        xp[:, c * FREE:(c + 1) * FREE] = flat[rows]
    return xs, xp


def kernel(x, base_weight, spline_weight, spline_scaler):
    x = np.asarray(x, dtype=np.float32)
    wq, bias = _prep_weights(np.asarray(base_weight), np.asarray(spline_weight),
                             np.asarray(spline_scaler))
    nc = build_nc()
    in_maps = []
    for k in range(NCORES):
        xs, xp = _prep_x(x[:, k * CLOC:(k + 1) * CLOC])
        in_maps.append({"xs": xs, "xp": xp, "wq": wq, "bias": bias})
    res = run_bass_kernel_spmd(nc, in_maps, list(range(NCORES)), **RUN_KWARGS)
    global LAST_EXEC_NS
    LAST_EXEC_NS = res.exec_time_ns
    outs = [np.asarray(r["out"]) for r in res.results]

    full = np.empty((B, 2304, OUT_C), dtype=np.float32)
    for k in range(NCORES):
        dev = outs[k].astype(np.float32).reshape(3, 3, OUT_C, B, CLOC, 4)
        rows = dev.transpose(3, 4, 0, 1, 5, 2).reshape(B, 288, OUT_C)
        full[:, 288 * k:288 * (k + 1), :] = rows
    return full.reshape(B, 128, 2304).reshape(B, 128, 48, 48)


# revision 5
# speedup vs baseline: 3.1615x; 3.1615x over previous
"""ConvKAN Trainium2 kernel (v5: h-major layout + bc-sliced early waves).

Decomposition (validated vs reference):
  out[(b, cin, kh, kw, q), oc] =
      sum_{func, jh, jw} Wf[oc, func, jh*48+jw] * F_func(x_pad[b, cin, 12q+jh+kh, jw+kw])
  where F_0 = silu and F_{1+g}(v) = spline cubes 4*r1^3 - r2^3 with
  t = |2.5 v + 3.5 - g|, r2 = max(2-t, 0), r1 = max(1-t, 0)
  (weights carry the -1/6 normalization).

Sharding: input channels cin split 8 ways (8 per core); core k produces
output rows [288k, 288k+288) of (B, 2304, OUT_C).

v5 scheduling (free dim stays (h=50, bc=128), b minor so rhs streams
contiguously):
- inputs on one HWDGE queue in consumption order
  (xs -> wq[silu] -> xp0 -> wq[p0] -> xp1 -> wq[p1] -> xp2 -> wq[p2]);
- silu and pass-0 planes are produced in bc-halves (strided sub-ops)
  and their matmul waves run N=256 b-slices, so the PE starts on half
  a plane; passes 1-2 run full N=512;
- start=True only on the first matmul touching each PSUM bank
  (per-element has_written semantics make partial-width accumulation
  sound); single stop on the last matmul per bank;
- group 8 runs right after group 0's bank drains; outputs are fp16.
"""

from contextlib import ExitStack

import numpy as np

import concourse.bass as bass
import concourse.bacc as bacc
import concourse.tile as tile
from concourse import mybir
from concourse.alu_op_type import AluOpType
from concourse.bass_utils import run_bass_kernel_spmd

AF = mybir.ActivationFunctionType
DT = mybir.dt

B, C, H, W = 16, 64, 48, 48
OUT_C = 128
NCORES = 8
CLOC = C // NCORES          # 8 input channels per core
BC = B * CLOC               # 128 (b, c) pairs per core
HP = 50                     # padded height
FREE = HP * BC              # 6400
NSP = 3                     # spline passes
NTILE = 3 * 6 + NSP * 3 * 12  # 126 lhsT tiles: (silu kw jj) + (pass kw jh)
FCH = 4                     # h chunks per activation pass
RUN_KWARGS = {}
LAST_EXEC_NS = None
N_WARMUP = 6                # HAM warm-up dummy matmuls

# engine-assignment knobs, [pass][chunk]: square ops on ACT vs DVE
S2_ACT = ((False, False, True, True),
          (True, True, False, False),
          (True, True, False, False))
S1F_ACT = ((True, True, True, True),
           (True, True, True, True),
           (True, True, True, True))
# which planes/waves are produced and consumed in bc-halves
HALF_SILU = True
HALF_P0 = True

V0 = (0.0, 0.0, -0.125, -2.875, -2.875, -0.125, 0.0, 0.0)  # slot value at x=0


def build_nc(fch: int = FCH) -> bass.Bass:
    nc = bacc.Bacc(None, target_bir_lowering=False, debug=True)
    xs = nc.declare_dram_parameter("xs", [128, FREE], DT.float16, isOutput=False)
    xp = nc.declare_dram_parameter("xp", [128, NSP * FREE], DT.float16,
                                   isOutput=False)
    wq = nc.declare_dram_parameter("wq", [128, NTILE * 128], DT.float16,
                                   isOutput=False)
    bias = nc.declare_dram_parameter("bias", [128, 8], DT.float32, isOutput=False)
    out = nc.declare_dram_parameter("out", [9, 128, 512], DT.float16, isOutput=True)

    fw = FREE // fch
    with ExitStack() as ctx:
        tc = ctx.enter_context(tile.TileContext(nc))
        wpool = ctx.enter_context(tc.tile_pool(name="w", bufs=1))
        fpool = ctx.enter_context(tc.tile_pool(name="f", bufs=2))
        psum_pool = ctx.enter_context(tc.tile_pool(name="ps", bufs=8, space="PSUM"))
        opool = ctx.enter_context(tc.tile_pool(name="o", bufs=4))

        bias_sb = wpool.tile([128, 8], DT.float32)
        nc.gpsimd.dma_start(bias_sb[:], bias[:])

        xs_sb = wpool.tile([128, FREE], DT.float16)
        wq_sb = wpool.tile([128, NTILE * 128], DT.float16)
        xp_sb = [wpool.tile([128, FREE], DT.float16, name=f"xp{c}", tag=f"xp{c}")
                 for c in range(NSP)]

        # input DMA stream in consumption order on the HWDGE queue
        for f in range(fch):
            sl = slice(f * fw, (f + 1) * fw)
            nc.sync.dma_start(xs_sb[:, sl], xs[:, sl])
        nc.sync.dma_start(wq_sb[:, :18 * 128], wq[:, :18 * 128])  # silu tiles
        for c in range(NSP):
            for f in range(fch):
                nc.sync.dma_start(xp_sb[c][:, f * fw:(f + 1) * fw],
                                  xp[:, c * FREE + f * fw:c * FREE + (f + 1) * fw])
            wsl = slice((18 + c * 36) * 128, (18 + (c + 1) * 36) * 128)
            nc.sync.dma_start(wq_sb[:, wsl], wq[:, wsl])

        ts_s = wpool.tile([128, FREE], DT.float16, name="tsS", tag="tsS")
        ts_t = [wpool.tile([128, FREE], DT.float16, name=f"ts{c}", tag=f"ts{c}")
                for c in range(NSP)]

        groups = [(kh, kw) for kh in range(3) for kw in range(3)]
        ps_tiles = {}
        for g in groups[:8]:
            ps_tiles[g] = psum_pool.tile([128, 512], DT.float32,
                                         name=f"ps_{g[0]}{g[1]}", tag="ps")
        # HAM warm-up into group-7's bank (cleared by its first start=True mm)
        warm = ps_tiles[groups[7]][:]
        for _ in range(N_WARMUP):
            nc.tensor.matmul(warm, xs_sb[:, 0:128], xs_sb[:, 512:1024],
                             start=True, stop=False)

        # h-row boundaries for bc-sliced sub-ops (fch must tile HP evenly
        # enough; 50/4 -> 12,13,12,13 rows)
        hbnd = [f * HP // fch for f in range(fch + 1)]

        def v3(tile_, cw, nb):
            # contiguous [128, cw] scratch viewed as [p, cw//nb, nb]
            return tile_[:, :cw].rearrange("p (h b) -> p h b", b=nb)

        # silu chain; bc-halved sub-ops when HALF_SILU
        xs_v = xs_sb[:].rearrange("p (h b) -> p h b", b=BC)
        ts_s_v = ts_s[:].rearrange("p (h b) -> p h b", b=BC)
        silu_bsplits = [(0, BC // 2), (BC // 2, BC)] if HALF_SILU else [(0, BC)]
        for b0, b1 in silu_bsplits:
            for f in range(fch):
                if b0 == 0 and b1 == BC:
                    sl = slice(f * fw, (f + 1) * fw)
                    nc.scalar.activation(ts_s[:, sl], xs_sb[:, sl], AF.Silu)
                else:
                    h0, h1 = hbnd[f], hbnd[f + 1]
                    nc.scalar.activation(ts_s_v[:, h0:h1, b0:b1],
                                         xs_v[:, h0:h1, b0:b1], AF.Silu)

        # spline chains
        for c in range(NSP):
            bias_ap = bias_sb[:, c:c + 1]
            xp_v = xp_sb[c][:].rearrange("p (h b) -> p h b", b=BC)
            ts_v = ts_t[c][:].rearrange("p (h b) -> p h b", b=BC)
            bsplits = ([(0, BC // 2), (BC // 2, BC)] if (c == 0 and HALF_P0)
                       else [(0, BC)])
            for b0, b1 in bsplits:
                for f in range(fch):
                    h0, h1 = hbnd[f], hbnd[f + 1]
                    nb = b1 - b0
                    split = nb < BC
                    cw = (h1 - h0) * nb if split else fw
                    mk = (lambda tag: v3(
                        fpool.tile([128, fw], DT.float16, name=tag, tag=tag),
                        cw, nb)
                        ) if split else (lambda tag: fpool.tile(
                        [128, fw], DT.float16, name=tag, tag=tag)[:, :cw])
                    src = (xp_v[:, h0:h1, b0:b1] if split
                           else xp_sb[c][:, f * fw:(f + 1) * fw])
                    dst = (ts_v[:, h0:h1, b0:b1] if split
                           else ts_t[c][:, f * fw:(f + 1) * fw])
                    t = mk("t")
                    nc.scalar.activation(t, src, AF.Abs, bias=bias_ap, scale=2.5)
                    nr2 = mk("nr2")
                    nc.vector.tensor_scalar(nr2, t, 2.0, 0.0,
                                            op0=AluOpType.subtract,
                                            op1=AluOpType.min)
                    nr1 = mk("nr1")
                    nc.vector.tensor_scalar(nr1, t, 1.0, 0.0,
                                            op0=AluOpType.subtract,
                                            op1=AluOpType.min)
                    s2 = mk("s2")
                    if S2_ACT[c][f]:
                        nc.scalar.activation(s2, nr2, AF.Square)
                    else:
                        nc.vector.tensor_tensor(s2, nr2, nr2, op=AluOpType.mult)
                    s1f = mk("s1f")
                    if S1F_ACT[c][f]:
                        nc.scalar.activation(s1f, nr1, AF.Square, scale=2.0)
                    else:
                        t4 = mk("t4")
                        nc.vector.tensor_scalar(t4, nr1, 2.0, 0.0,
                                                op0=AluOpType.mult,
                                                op1=AluOpType.bypass)
                        nc.vector.tensor_tensor(s1f, t4, t4, op=AluOpType.mult)
                    c2n = mk("c2n")
                    nc.vector.tensor_tensor(c2n, s2, nr2, op=AluOpType.mult)
                    cn1 = mk("cn1")
                    nc.vector.tensor_tensor(cn1, s1f, nr1, op=AluOpType.mult)
                    nc.vector.tensor_tensor(dst, c2n, cn1, op=AluOpType.subtract)

        def emit_mm(g, seq, b0=0, b1=BC, start=False, stop=False):
            kh, kw = g
            kind, c, j = seq
            if kind == "S":
                idx = kw * 6 + j
                src = ts_s
            else:
                idx = 18 + (c * 3 + kw) * 12 + j
                src = ts_t[c]
            lhsT = wq_sb[:, idx * 128:(idx + 1) * 128]
            h0 = kh + j
            rhs = src[:].rearrange("p (h b) -> p h b", b=BC)[:, h0:h0 + 37:12,
                                                             b0:b1]
            ps3 = ps_tiles[g][:].rearrange("p (q b) -> p q b", b=BC)[:, :, b0:b1]
            nc.tensor.matmul(ps3, lhsT, rhs, start=start, stop=stop)

        def drain(g):
            ob = opool.tile([128, 512], DT.float16)
            # adds the constant contribution of the removed w_pad 0/49 slots
            nc.scalar.activation(ob[:], ps_tiles[g][:], AF.Identity,
                                 bias=bias_sb[:, 4 + g[1]:5 + g[1]])
            nc.sync.dma_start(out[g[0] * 3 + g[1]], ob[:])

        wave = groups[:8]
        silu_seqs = [("S", 0, j) for j in range(6)]
        pass_seqs = [[("P", c, j) for j in range(12)] for c in range(NSP)]
        all_seqs = silu_seqs + pass_seqs[0] + pass_seqs[1] + pass_seqs[2]

        for hi, (b0, b1) in enumerate(silu_bsplits):
            for j, s in enumerate(silu_seqs):
                for g in wave:
                    emit_mm(g, s, b0, b1, start=(hi == 0 and j == 0))
        p0_halves = [(0, BC // 2), (BC // 2, BC)] if HALF_P0 else [(0, BC)]
        for b0, b1 in p0_halves:
            for s in pass_seqs[0]:
                for g in wave:
                    emit_mm(g, s, b0, b1)
        for s in pass_seqs[1]:
            for g in wave:
                emit_mm(g, s)
        # pass 2: group 0 first, drain it, then g8's full run, then the rest
        for i, s in enumerate(pass_seqs[2]):
            emit_mm(wave[0], s, stop=(i == 11))
        drain(wave[0])
        g8 = groups[8]
        ps_tiles[g8] = psum_pool.tile([128, 512], DT.float32, name="ps_22",
                                      tag="ps")
        for i, s in enumerate(all_seqs):
            emit_mm(g8, s, start=(i == 0), stop=(i == len(all_seqs) - 1))
        drain(g8)
        for g in wave[1:]:
            for i, s in enumerate(pass_seqs[2]):
                emit_mm(g, s, stop=(i == 11))
            drain(g)
    nc.compile()
    return nc


def _prep_weights(base_weight, spline_weight, spline_scaler):
    # Wf[oc, func, jj]: func 0 = silu weights, 1+g = scaled spline / -6
    wf = np.empty((OUT_C, 9, 576), dtype=np.float64)
    wf[:, 0, :] = base_weight
    wf[:, 1:, :] = np.moveaxis(
        spline_weight.astype(np.float64)
        * spline_scaler.astype(np.float64)[..., None] / -6.0, -1, 1)
    w4 = wf.reshape(OUT_C, 9, 12, 48)
    wq = np.zeros((128, NTILE, OUT_C), dtype=np.float64)
    for kw in range(3):
        for jj in range(6):  # silu tiles
            idx = kw * 6 + jj
            for p in range(96):
                s, wp = p // 48, 1 + p % 48
                jw = wp - kw
                if 0 <= jw < 48:
                    wq[p, idx, :] = w4[:, 0, jj + 6 * s, jw]
    for c in range(NSP):
        for kw in range(3):
            for jh in range(12):
                idx = 18 + (c * 3 + kw) * 12 + jh
                for p in range(128):
                    flat = 128 * c + p
                    g, wp = flat // 48, 1 + flat % 48
                    jw = wp - kw
                    if 0 <= jw < 48:
                        wq[p, idx, :] = w4[:, 1 + g, jh, jw]
    wq = wq.reshape(128, NTILE * 128).astype(np.float16)

    bias = np.zeros((128, 8), dtype=np.float32)
    for c in range(NSP):
        for p in range(128):
            bias[p, c] = 3.5 - (128 * c + p) // 48
    # drain-time constant for removed w_pad 0 (kw=0) / 49 (kw=2) slots
    for g in range(8):
        bias[:, 4] += V0[g] * w4[:, 1 + g, :, 0].sum(axis=1)
        bias[:, 6] += V0[g] * w4[:, 1 + g, :, 47].sum(axis=1)
    return wq, bias


def _prep_x(x_slice):
    # x_slice: (B, CLOC, 48, 48) -> (xs [128, FREE], xp [128, NSP*FREE]) fp16
    plane = np.zeros((HP, HP, BC), dtype=np.float32)
    plane[1:49, 1:49, :] = np.ascontiguousarray(
        x_slice.transpose(3, 2, 0, 1)).reshape(48, 48, BC)
    flat = plane.reshape(HP, FREE)          # [w_pad, h*bc]
    sh6 = np.zeros_like(plane)              # h-shift by 6
    sh6[:, 0:44, :] = plane[:, 6:50, :]
    flat6 = sh6.reshape(HP, FREE)

    xs = np.zeros((128, FREE), dtype=np.float16)
    xs[0:48] = flat[1:49]
    xs[48:96] = flat6[1:49]
    xp = np.empty((128, NSP * FREE), dtype=np.float16)
    for c in range(NSP):
        rows = [1 + (128 * c + p) % 48 for p in range(128)]
        xp[:, c * FREE:(c + 1) * FREE] = flat[rows]
    return xs, xp


def kernel(x, base_weight, spline_weight, spline_scaler):
    x = np.asarray(x, dtype=np.float32)
    wq, bias = _prep_weights(np.asarray(base_weight), np.asarray(spline_weight),
                             np.asarray(spline_scaler))
    nc = build_nc()
    in_maps = []
    for k in range(NCORES):
        xs, xp = _prep_x(x[:, k * CLOC:(k + 1) * CLOC])
        in_maps.append({"xs": xs, "xp": xp, "wq": wq, "bias": bias})
    res = run_bass_kernel_spmd(nc, in_maps, list(range(NCORES)), **RUN_KWARGS)
    global LAST_EXEC_NS
    LAST_EXEC_NS = res.exec_time_ns
    outs = [np.asarray(r["out"]) for r in res.results]

    full = np.empty((B, 2304, OUT_C), dtype=np.float32)
    for k in range(NCORES):
        dev = outs[k].astype(np.float32).reshape(3, 3, OUT_C, 4, B, CLOC)
        rows = dev.transpose(4, 5, 0, 1, 3, 2).reshape(B, 288, OUT_C)
        full[:, 288 * k:288 * (k + 1), :] = rows
    return full.reshape(B, 128, 2304).reshape(B, 128, 48, 48)


# revision 6
# speedup vs baseline: 3.3367x; 1.0554x over previous
"""ConvKAN Trainium2 kernel (v6: mixed-layout planes, early half waves).

Decomposition (validated vs reference):
  out[(b, cin, kh, kw, q), oc] =
      sum_{func, jh, jw} Wf[oc, func, jh*48+jw] * F_func(x_pad[b, cin, 12q+jh+kh, jw+kw])
  where F_0 = silu and F_{1+g}(v) = spline cubes 4*r1^3 - r2^3 with
  t = |2.5 v + 3.5 - g|, r2 = max(2-t, 0), r1 = max(1-t, 0)
  (weights carry the -1/6 normalization).

Sharding: input channels cin split 8 ways (8 per core); core k produces
output rows [288k, 288k+288) of (B, 2304, OUT_C).

v6 scheduling:
- silu and pass-0 planes (and their x inputs) are stored bc-half-blocked
  [p, (x:2, h:50, b':64)] so both the elementwise chain (flat chunks, no
  strided penalty) and the matmul waves (N=256 per half, contiguous rhs)
  can proceed on half a plane; passes 1-2 keep the classic [p, (h, b)]
  layout and full N=512 matmuls (N=256 everywhere would be
  LDWEIGHTS-bound).  PSUM keeps the classic (q, b) layout; half waves
  write the b-slice, so both widths accumulate consistently.
- inputs stream on one HWDGE queue in consumption order:
  xs[h0] -> wq[silu] -> xs[h1] -> xp0 -> wq[p0] -> xp1 -> wq[p1] -> xp2 -> wq[p2]
- start=True only on the first matmul touching each PSUM bank; single
  stop on the last (per-element has_written semantics make mixed-width
  accumulation sound).
- group 8 runs right after group 0's bank drains; outputs are fp16;
  matmul triples share lhsT (kh inner) where possible.
"""

from contextlib import ExitStack

import numpy as np

import concourse.bass as bass
import concourse.bacc as bacc
import concourse.tile as tile
from concourse import mybir
from concourse.alu_op_type import AluOpType
from concourse.bass_utils import run_bass_kernel_spmd

AF = mybir.ActivationFunctionType
DT = mybir.dt

B, C, H, W = 16, 64, 48, 48
OUT_C = 128
NCORES = 8
CLOC = C // NCORES          # 8 input channels per core
BC = B * CLOC               # 128 (b, c) pairs per core
HP = 50                     # padded height
FREE = HP * BC              # 6400
NSP = 3                     # spline passes
NTILE = 3 * 6 + NSP * 3 * 12  # 126 lhsT tiles: (silu kw jj) + (pass kw jh)
FCH = 4                     # chunks per activation pass (1600 cols each)
RUN_KWARGS = {}
LAST_EXEC_NS = None
N_WARMUP = 8                # HAM warm-up dummy matmuls

# engine-assignment knobs, [pass][chunk]: square ops on ACT vs DVE
S2_ACT = ((False, False, True, True),
          (True, True, False, False),
          (True, True, False, False))
S1F_ACT = ((True, True, True, True),
           (True, True, True, True),
           (True, True, True, True))

V0 = (0.0, 0.0, -0.125, -2.875, -2.875, -0.125, 0.0, 0.0)  # slot value at x=0


def build_nc(fch: int = FCH) -> bass.Bass:
    nc = bacc.Bacc(None, target_bir_lowering=False, debug=True)
    xs = nc.declare_dram_parameter("xs", [128, FREE], DT.float16, isOutput=False)
    xp = nc.declare_dram_parameter("xp", [128, NSP * FREE], DT.float16,
                                   isOutput=False)
    wq = nc.declare_dram_parameter("wq", [128, NTILE * 128], DT.float16,
                                   isOutput=False)
    bias = nc.declare_dram_parameter("bias", [128, 8], DT.float32, isOutput=False)
    out = nc.declare_dram_parameter("out", [9, 128, 512], DT.float16, isOutput=True)

    fw = FREE // fch
    with ExitStack() as ctx:
        tc = ctx.enter_context(tile.TileContext(nc))
        wpool = ctx.enter_context(tc.tile_pool(name="w", bufs=1))
        fpool = ctx.enter_context(tc.tile_pool(name="f", bufs=2))
        psum_pool = ctx.enter_context(tc.tile_pool(name="ps", bufs=8, space="PSUM"))
        opool = ctx.enter_context(tc.tile_pool(name="o", bufs=4))

        bias_sb = wpool.tile([128, 8], DT.float32)
        nc.gpsimd.dma_start(bias_sb[:], bias[:])

        xs_sb = wpool.tile([128, FREE], DT.float16)
        wq_sb = wpool.tile([128, NTILE * 128], DT.float16)
        xp_sb = [wpool.tile([128, FREE], DT.float16, name=f"xp{c}", tag=f"xp{c}")
                 for c in range(NSP)]

        # input DMA stream in consumption order on the HWDGE queue
        def dma_chunks(dst, src_base, cs):
            for f in cs:
                nc.sync.dma_start(dst[:, f * fw:(f + 1) * fw],
                                  xp[:, src_base + f * fw:src_base + (f + 1) * fw]
                                  if src_base is not None else
                                  xs[:, f * fw:(f + 1) * fw])

        dma_chunks(xs_sb, None, (0, 1))
        nc.sync.dma_start(wq_sb[:, :18 * 128], wq[:, :18 * 128])  # silu tiles
        dma_chunks(xs_sb, None, (2, 3))
        for c in range(NSP):
            dma_chunks(xp_sb[c], c * FREE, range(fch))
            wsl = slice((18 + c * 36) * 128, (18 + (c + 1) * 36) * 128)
            nc.sync.dma_start(wq_sb[:, wsl], wq[:, wsl])

        ts_s = wpool.tile([128, FREE], DT.float16, name="tsS", tag="tsS")
        ts_t = [wpool.tile([128, FREE], DT.float16, name=f"ts{c}", tag=f"ts{c}")
                for c in range(NSP)]

        groups = [(kh, kw) for kh in range(3) for kw in range(3)]
        ps_tiles = {}
        for g in groups[:8]:
            ps_tiles[g] = psum_pool.tile([128, 512], DT.float32,
                                         name=f"ps_{g[0]}{g[1]}", tag="ps")
        # HAM warm-up into group-7's bank (cleared by its first start=True mm)
        warm = ps_tiles[groups[7]][:]
        for _ in range(N_WARMUP):
            nc.tensor.matmul(warm, xs_sb[:, 0:128], xs_sb[:, 512:1024],
                             start=True, stop=False)

        # silu chain: one ACT op per chunk (chunks 0,1 = half 0)
        for f in range(fch):
            sl = slice(f * fw, (f + 1) * fw)
            nc.scalar.activation(ts_s[:, sl], xs_sb[:, sl], AF.Silu)

        # spline chains (flat chunks in every layout)
        for c in range(NSP):
            bias_ap = bias_sb[:, c:c + 1]
            for f in range(fch):
                sl = slice(f * fw, (f + 1) * fw)
                t = fpool.tile([128, fw], DT.float16, tag="t")
                nc.scalar.activation(t[:], xp_sb[c][:, sl], AF.Abs,
                                     bias=bias_ap, scale=2.5)
                nr2 = fpool.tile([128, fw], DT.float16, tag="nr2")  # -r2
                nc.vector.tensor_scalar(nr2[:], t[:], 2.0, 0.0,
                                        op0=AluOpType.subtract, op1=AluOpType.min)
                nr1 = fpool.tile([128, fw], DT.float16, tag="nr1")  # -r1
                nc.vector.tensor_scalar(nr1[:], t[:], 1.0, 0.0,
                                        op0=AluOpType.subtract, op1=AluOpType.min)
                s2 = fpool.tile([128, fw], DT.float16, tag="s2")    # r2^2
                if S2_ACT[c][f]:
                    nc.scalar.activation(s2[:], nr2[:], AF.Square)
                else:
                    nc.vector.tensor_tensor(s2[:], nr2[:], nr2[:],
                                            op=AluOpType.mult)
                s1f = fpool.tile([128, fw], DT.float16, tag="s1f")  # 4 r1^2
                if S1F_ACT[c][f]:
                    nc.scalar.activation(s1f[:], nr1[:], AF.Square, scale=2.0)
                else:
                    t4 = fpool.tile([128, fw], DT.float16, tag="t4")
                    nc.vector.tensor_scalar(t4[:], nr1[:], 2.0, 0.0,
                                            op0=AluOpType.mult,
                                            op1=AluOpType.bypass)
                    nc.vector.tensor_tensor(s1f[:], t4[:], t4[:],
                                            op=AluOpType.mult)
                c2n = fpool.tile([128, fw], DT.float16, tag="c2n")  # -r2^3
                nc.vector.tensor_tensor(c2n[:], s2[:], nr2[:], op=AluOpType.mult)
                cn1 = fpool.tile([128, fw], DT.float16, tag="cn1")  # -4 r1^3
                nc.vector.tensor_tensor(cn1[:], s1f[:], nr1[:], op=AluOpType.mult)
                nc.vector.tensor_tensor(ts_t[c][:, sl], c2n[:], cn1[:],
                                        op=AluOpType.subtract)

        def emit_mm(g, seq, x=None, start=False, stop=False):
            # x = None: classic full-width N=512; x in (0,1): blocked half
            kh, kw = g
            kind, c, j = seq
            if kind == "S":
                idx = kw * 6 + j
                src = ts_s
            else:
                idx = 18 + (c * 3 + kw) * 12 + j
                src = ts_t[c]
            lhsT = wq_sb[:, idx * 128:(idx + 1) * 128]
            h0 = kh + j
            ps = ps_tiles[g][:].rearrange("p (q b) -> p q b", b=BC)
            if x is None:
                rhs = src[:].rearrange("p (h b) -> p h b", b=BC)[:, h0:h0 + 37:12]
                out_ap = ps
            else:
                rhs = src[:].rearrange("p (x h b) -> p x h b", x=2, b=64)[
                    :, x, h0:h0 + 37:12, :]
                out_ap = ps[:, :, 64 * x:64 * (x + 1)]
            nc.tensor.matmul(out_ap, lhsT, rhs, start=start, stop=stop)

        def drain(g):
            ob = opool.tile([128, 512], DT.float16)
            # adds the constant contribution of the removed w_pad 0/49 slots
            nc.scalar.activation(ob[:], ps_tiles[g][:], AF.Identity,
                                 bias=bias_sb[:, 4 + g[1]:5 + g[1]])
            nc.sync.dma_start(out[g[0] * 3 + g[1]], ob[:])

        wave = groups[:8]
        silu_seqs = [("S", 0, j) for j in range(6)]
        pass_seqs = [[("P", c, j) for j in range(12)] for c in range(NSP)]

        def kworder(include_g8=False):
            gs = groups[:9] if include_g8 else groups[:8]
            return sorted(gs, key=lambda g: (g[1], g[0]))  # kw major, kh inner

        # silu + pass0 waves: per bc-half, kh-inner triples share lhsT
        for x in (0, 1):
            for j, s in enumerate(silu_seqs):
                for g in kworder():
                    emit_mm(g, s, x=x, start=(x == 0 and j == 0))
        for x in (0, 1):
            for s in pass_seqs[0]:
                for g in kworder():
                    emit_mm(g, s, x=x)
        # pass 1: full width
        for s in pass_seqs[1]:
            for g in kworder():
                emit_mm(g, s)
        # pass 2: group 0 first, drain it, then g8's full run, then the rest
        for i, s in enumerate(pass_seqs[2]):
            emit_mm(wave[0], s, stop=(i == 11))
        drain(wave[0])
        g8 = groups[8]
        ps_tiles[g8] = psum_pool.tile([128, 512], DT.float32, name="ps_22",
                                      tag="ps")
        first = True
        for s in silu_seqs + pass_seqs[0]:
            for x in (0, 1):
                emit_mm(g8, s, x=x, start=first)
                first = False
        for s in pass_seqs[1]:
            emit_mm(g8, s)
        for i, s in enumerate(pass_seqs[2]):
            emit_mm(g8, s, stop=(i == 11))
        drain(g8)
        for g in kworder()[1:] if False else [g for g in kworder() if g != wave[0]]:
            for i, s in enumerate(pass_seqs[2]):
                emit_mm(g, s, stop=(i == 11))
            drain(g)
    nc.compile()
    return nc


def _prep_weights(base_weight, spline_weight, spline_scaler):
    # Wf[oc, func, jj]: func 0 = silu weights, 1+g = scaled spline / -6
    wf = np.empty((OUT_C, 9, 576), dtype=np.float64)
    wf[:, 0, :] = base_weight
    wf[:, 1:, :] = np.moveaxis(
        spline_weight.astype(np.float64)
        * spline_scaler.astype(np.float64)[..., None] / -6.0, -1, 1)
    w4 = wf.reshape(OUT_C, 9, 12, 48)
    wq = np.zeros((128, NTILE, OUT_C), dtype=np.float64)
    for kw in range(3):
        for jj in range(6):  # silu tiles
            idx = kw * 6 + jj
            for p in range(96):
                s, wp = p // 48, 1 + p % 48
                jw = wp - kw
                if 0 <= jw < 48:
                    wq[p, idx, :] = w4[:, 0, jj + 6 * s, jw]
    for c in range(NSP):
        for kw in range(3):
            for jh in range(12):
                idx = 18 + (c * 3 + kw) * 12 + jh
                for p in range(128):
                    flat = 128 * c + p
                    g, wp = flat // 48, 1 + flat % 48
                    jw = wp - kw
                    if 0 <= jw < 48:
                        wq[p, idx, :] = w4[:, 1 + g, jh, jw]
    wq = wq.reshape(128, NTILE * 128).astype(np.float16)

    bias = np.zeros((128, 8), dtype=np.float32)
    for c in range(NSP):
        for p in range(128):
            bias[p, c] = 3.5 - (128 * c + p) // 48
    # drain-time constant for removed w_pad 0 (kw=0) / 49 (kw=2) slots
    for g in range(8):
        bias[:, 4] += V0[g] * w4[:, 1 + g, :, 0].sum(axis=1)
        bias[:, 6] += V0[g] * w4[:, 1 + g, :, 47].sum(axis=1)
    return wq, bias


def _prep_x(x_slice):
    # x_slice: (B, CLOC, 48, 48) -> (xs [128, FREE], xp [128, NSP*FREE]) fp16
    # xs and xp[0] are bc-half-blocked: col = x*3200 + h*64 + b'
    # xp[1], xp[2] classic: col = h*128 + b
    plane = np.zeros((HP, HP, BC), dtype=np.float32)
    plane[1:49, 1:49, :] = np.ascontiguousarray(
        x_slice.transpose(3, 2, 0, 1)).reshape(48, 48, BC)
    flat = plane.reshape(HP, FREE)          # [w_pad, h*bc]
    sh6 = np.zeros_like(plane)              # h-shift by 6
    sh6[:, 0:44, :] = plane[:, 6:50, :]
    flat6 = sh6.reshape(HP, FREE)

    def blocked(a):  # [rows, h*bc] -> [rows, (x h b')]
        r = a.reshape(-1, HP, 2, 64)
        return np.ascontiguousarray(r.transpose(0, 2, 1, 3)).reshape(-1, FREE)

    xs = np.zeros((128, FREE), dtype=np.float16)
    xs[0:48] = blocked(flat[1:49])
    xs[48:96] = blocked(flat6[1:49])
    xp = np.empty((128, NSP * FREE), dtype=np.float16)
    for c in range(NSP):
        rows = [1 + (128 * c + p) % 48 for p in range(128)]
        body = flat[rows]
        if c == 0:
            body = blocked(body)
        xp[:, c * FREE:(c + 1) * FREE] = body
    return xs, xp


def kernel(x, base_weight, spline_weight, spline_scaler):
    x = np.asarray(x, dtype=np.float32)
    wq, bias = _prep_weights(np.asarray(base_weight), np.asarray(spline_weight),
                             np.asarray(spline_scaler))
    nc = build_nc()
    in_maps = []
    for k in range(NCORES):
        xs, xp = _prep_x(x[:, k * CLOC:(k + 1) * CLOC])
        in_maps.append({"xs": xs, "xp": xp, "wq": wq, "bias": bias})
    res = run_bass_kernel_spmd(nc, in_maps, list(range(NCORES)), **RUN_KWARGS)
    global LAST_EXEC_NS
    LAST_EXEC_NS = res.exec_time_ns
    outs = [np.asarray(r["out"]) for r in res.results]

    full = np.empty((B, 2304, OUT_C), dtype=np.float32)
    for k in range(NCORES):
        dev = outs[k].astype(np.float32).reshape(3, 3, OUT_C, 4, B, CLOC)
        rows = dev.transpose(4, 5, 0, 1, 3, 2).reshape(B, 288, OUT_C)
        full[:, 288 * k:288 * (k + 1), :] = rows
    return full.reshape(B, 128, 2304).reshape(B, 128, 48, 48)
